# revision 1
# baseline (speedup 1.0000x reference)
"""DecoderRNN (embedding -> 2x GRU(shared weights) -> vocab Linear -> log_softmax)
as a Bass/Tile kernel on 8 Trainium2 NeuronCores.

Sharding (tensor-parallel over vocab + hidden):
  - Embedding: the single needed row emb[token] is sliced host-side (the rest
    of the 206MB table is never touched) and replicated across partitions.
  - GRU: hidden dim 1024 sharded 8-way -> 128 units/core. Each core holds the
    [3*128, 1024] slices of w_ih/w_hh (bias packed as an extra input column).
    Full h is rebuilt between layers with a tiny AllGather + broadcast DMA.
  - Output Linear: vocab (50257, padded to 51200) sharded 8-way -> 6400/core,
    laid out [128 partitions x 50 slots x 1025] (bias packed in column 1024).
    Each dot-product row is a fused multiply+reduce (scalar_tensor_tensor with
    accum_out) on the Vector engine.
  - log_softmax: local max/sum-exp per core, one AllGather of 2 floats/core,
    closed-form combine, then a single bias-add pass over the local logits.

All compute is f32 on device; host only slices/replicates/concatenates.
"""

import numpy as np

from concourse import bacc, tile, mybir, bass_utils

NHID = 1024
NOUT = 50257
N_CORES = 8
P = 128
HP = NHID // N_CORES        # 128 hidden units per core
SLOTS = 50                  # vocab rows per partition per core
VSHARD = P * SLOTS          # 6400 vocab entries per core
VPAD = VSHARD * N_CORES     # 51200 padded vocab
F = NHID + 1                # 1025: weights with bias packed in last column
CHUNK = 5                   # vocab slots per streamed W tile
NCHUNK = SLOTS // CHUNK
WBUFS = 4                   # W stream double-buffering depth
NEG_BIG = -1.0e30

F32 = mybir.dt.float32
Alu = mybir.AluOpType
Act = mybir.ActivationFunctionType

_CACHE = {}
LAST_EXEC_NS = None


def _stt_dot(nc, spool, w_ap, x_ap, acc_ap):
    """acc[p] = sum_f w[p, f] * x[p, f] (fused one-pass DVE op)."""
    prod = spool.tile([P, F], F32, tag="prod")
    nc.vector.scalar_tensor_tensor(
        out=prod[:],
        in0=w_ap,
        scalar=1.0,
        in1=x_ap,
        op0=Alu.mult,
        op1=Alu.mult,
        accum_out=acc_ap,
    )


def _build():
    nc = bacc.Bacc(
        "TRN2", target_bir_lowering=False, debug=False, num_devices=N_CORES
    )

    xe_d = nc.dram_tensor("xe", [P, F], F32, kind="ExternalInput")
    he_d = nc.dram_tensor("he", [P, F], F32, kind="ExternalInput")
    hsl_d = nc.dram_tensor("hsl", [P, 1], F32, kind="ExternalInput")
    wih_d = nc.dram_tensor("wih", [P, 3, F], F32, kind="ExternalInput")
    whh_d = nc.dram_tensor("whh", [P, 3, F], F32, kind="ExternalInput")
    wout_d = nc.dram_tensor("wout", [P, SLOTS, F], F32, kind="ExternalInput")
    logp_d = nc.dram_tensor("logp", [P, SLOTS], F32, kind="ExternalOutput")
    hout_d = nc.dram_tensor("hout", [P, 1], F32, kind="ExternalOutput")

    groups = [list(range(N_CORES))]

    with tile.TileContext(nc) as tc:
        with (
            tc.tile_pool(name="persist", bufs=1) as pp,
            tc.tile_pool(name="small", bufs=2) as sp,
            tc.tile_pool(name="scratch", bufs=3) as spool,
            tc.tile_pool(name="wstream", bufs=WBUFS) as wpool,
            tc.tile_pool(name="dram", bufs=1, space="DRAM") as dram,
        ):
            # ---- persistent tiles ----
            wih_t = pp.tile([P, 3, F], F32, tag="wih")
            whh_t = pp.tile([P, 3, F], F32, tag="whh")
            xe_t = pp.tile([P, F], F32, tag="xe")
            he_t = pp.tile([P, F], F32, tag="he")
            ge_t = pp.tile([P, F], F32, tag="ge")    # layer-2 x=h=h1 bcast
            oe_t = pp.tile([P, F], F32, tag="oe")    # h2 bcast for vocab dots
            hsl_t = pp.tile([P, 1], F32, tag="hsl")
            logits = pp.tile([P, SLOTS], F32, tag="logits")

            i_wih = nc.sync.dma_start(wih_t[:], wih_d[:, :, :])
            i_whh = nc.sync.dma_start(whh_t[:], whh_d[:, :, :])
            i_xe = nc.sync.dma_start(xe_t[:], xe_d[:, :])
            i_he = nc.sync.dma_start(he_t[:], he_d[:, :])
            nc.sync.dma_start(hsl_t[:], hsl_d[:, :])
            gate_insts = [i_wih, i_whh, i_xe, i_he]

            # ones column for the rebuilt h tiles
            nc.vector.memset(ge_t[:, NHID:], 1.0)
            nc.vector.memset(oe_t[:, NHID:], 1.0)

            # x = relu(emb[token])  (leaves the packed 1.0 column alone)
            nc.scalar.activation(xe_t[:, :NHID], xe_t[:, :NHID], Act.Relu)

            def gru_layer(x_t, h_t, hsl_ap, lidx):
                gi, gh = [], []
                for g in range(3):
                    a = sp.tile([P, 1], F32, tag=f"gi{g}")
                    _stt_dot(nc, spool, wih_t[:, g, :], x_t[:], a[:])
                    gi.append(a)
                for g in range(3):
                    a = sp.tile([P, 1], F32, tag=f"gh{g}")
                    _stt_dot(nc, spool, whh_t[:, g, :], h_t[:], a[:])
                    gh.append(a)
                r = sp.tile([P, 1], F32, tag="r")
                z = sp.tile([P, 1], F32, tag="z")
                nc.scalar.activation(r[:], gi[0][:], Act.Sigmoid, bias=gh[0][:])
                nc.scalar.activation(z[:], gi[1][:], Act.Sigmoid, bias=gh[1][:])
                tn = sp.tile([P, 1], F32, tag="tn")
                # tn = r * h_n + i_n
                nc.vector.scalar_tensor_tensor(
                    out=tn[:], in0=gh[2][:], scalar=r[:], in1=gi[2][:],
                    op0=Alu.mult, op1=Alu.add,
                )
                n = sp.tile([P, 1], F32, tag="n")
                nc.scalar.activation(n[:], tn[:], Act.Tanh)
                d = sp.tile([P, 1], F32, tag="d")
                nc.vector.tensor_tensor(d[:], hsl_ap, n[:], Alu.subtract)
                hnew = sp.tile([P, 1], F32, tag=f"hnew{lidx}")
                # hnew = z * (h - n) + n
                nc.vector.scalar_tensor_tensor(
                    out=hnew[:], in0=d[:], scalar=z[:], in1=n[:],
                    op0=Alu.mult, op1=Alu.add,
                )
                return hnew

            def exchange(hsl_tile, tagn, dest_tile):
                cc_in = dram.tile([HP], F32, tag=f"ccin{tagn}")
                cc_out = dram.tile([NHID], F32, tag=f"ccout{tagn}")
                nc.sync.dma_start(cc_in[:], hsl_tile[:, 0])
                nc.gpsimd.collective_compute(
                    "AllGather", Alu.bypass, replica_groups=groups,
                    ins=[cc_in[:].opt()], outs=[cc_out[:].opt()],
                )
                nc.sync.dma_start(
                    dest_tile[:, :NHID], cc_out[None, :].to_broadcast([P, NHID])
                )

            h1 = gru_layer(xe_t, he_t, hsl_t[:], 0)
            exchange(h1, 0, ge_t)
            h2 = gru_layer(ge_t, ge_t, h1[:], 1)
            nc.sync.dma_start(hout_d[:, :], h2[:])
            exchange(h2, 1, oe_t)

            # ---- vocab dot products, streaming W ----
            for t in range(NCHUNK):
                wt = wpool.tile([P, CHUNK, F], F32, tag="wt")
                dma = nc.sync.dma_start(
                    wt[:], wout_d[:, t * CHUNK:(t + 1) * CHUNK, :]
                )
                if t < WBUFS:
                    for g in gate_insts:
                        tile.add_dep_helper(
                            dma.ins, g.ins, sync=True,
                            reason="let GRU inputs land first",
                        )
                for j in range(CHUNK):
                    k = t * CHUNK + j
                    _stt_dot(nc, spool, wt[:, j, :], oe_t[:], logits[:, k:k + 1])

            # ---- log_softmax ----
            rowmax = sp.tile([P, 1], F32, tag="rowmax")
            nc.vector.tensor_reduce(
                rowmax[:], logits[:], mybir.AxisListType.X, Alu.max
            )
            dmax = dram.tile([P], F32, tag="dmax")
            nc.sync.dma_start(dmax[:], rowmax[:, 0])
            maxrow = sp.tile([1, P], F32, tag="maxrow")
            nc.sync.dma_start(maxrow[:], dmax[None, :])
            m = sp.tile([1, 1], F32, tag="m")
            nc.vector.tensor_reduce(m[:], maxrow[:], mybir.AxisListType.X, Alu.max)
            negm = sp.tile([1, 1], F32, tag="negm")
            nc.vector.tensor_scalar_mul(negm[:], m[:], -1.0)
            dnegm = dram.tile([1], F32, tag="dnegm")
            nc.sync.dma_start(dnegm[:], negm[0, :])
            negm_b = sp.tile([P, 1], F32, tag="negm_b")
            nc.sync.dma_start(negm_b[:], dnegm[None, :].to_broadcast([P, 1]))

            ex = sp.tile([P, SLOTS], F32, tag="ex")
            rowsum = sp.tile([P, 1], F32, tag="rowsum")
            nc.scalar.activation(
                ex[:], logits[:], Act.Exp, bias=negm_b[:], accum_out=rowsum[:]
            )
            dsum = dram.tile([P], F32, tag="dsum")
            nc.sync.dma_start(dsum[:], rowsum[:, 0])
            sumrow = sp.tile([1, P], F32, tag="sumrow")
            nc.sync.dma_start(sumrow[:], dsum[None, :])
            s = sp.tile([1, 1], F32, tag="s")
            nc.vector.tensor_reduce(s[:], sumrow[:], mybir.AxisListType.X, Alu.add)

            # stats AllGather: (m, s) per core -> [8, 2]
            st2 = dram.tile([2], F32, tag="st2")
            st16 = dram.tile([2 * N_CORES], F32, tag="st16")
            nc.sync.dma_start(st2[0:1], m[0, :])
            nc.sync.dma_start(st2[1:2], s[0, :])
            nc.gpsimd.collective_compute(
                "AllGather", Alu.bypass, replica_groups=groups,
                ins=[st2[:].opt()], outs=[st16[:].opt()],
            )
            strow = sp.tile([1, N_CORES, 2], F32, tag="strow")
            nc.sync.dma_start(strow[:], st16[:])
            m_vals = strow[:, :, 0]
            s_vals = strow[:, :, 1]
            gm = sp.tile([1, 1], F32, tag="gm")
            nc.vector.tensor_reduce(gm[:], m_vals, mybir.AxisListType.X, Alu.max)
            neggm = sp.tile([1, 1], F32, tag="neggm")
            nc.vector.tensor_scalar_mul(neggm[:], gm[:], -1.0)
            e8 = sp.tile([1, N_CORES], F32, tag="e8")
            nc.scalar.activation(e8[:], m_vals, Act.Exp, bias=neggm[:])
            se8 = sp.tile([1, N_CORES], F32, tag="se8")
            nc.vector.tensor_tensor(se8[:], e8[:], s_vals, Alu.mult)
            gs = sp.tile([1, 1], F32, tag="gs")
            nc.vector.tensor_reduce(gs[:], se8[:], mybir.AxisListType.X, Alu.add)
            lgs = sp.tile([1, 1], F32, tag="lgs")
            nc.scalar.activation(lgs[:], gs[:], Act.Ln)
            negc = sp.tile([1, 1], F32, tag="negc")
            # negc = -gm - log(gs)
            nc.vector.tensor_tensor(negc[:], neggm[:], lgs[:], Alu.subtract)
            dnegc = dram.tile([1], F32, tag="dnegc")
            nc.sync.dma_start(dnegc[:], negc[0, :])
            negc_b = sp.tile([P, 1], F32, tag="negc_b")
            nc.sync.dma_start(negc_b[:], dnegc[None, :].to_broadcast([P, 1]))

            lp = sp.tile([P, SLOTS], F32, tag="lp")
            nc.scalar.activation(lp[:], logits[:], Act.Identity, bias=negc_b[:])
            nc.sync.dma_start(logp_d[:, :], lp[:])

    nc.compile()
    return nc


def kernel(token, hidden, emb, w_ih, w_hh, b_ih, b_hh, W_out, b_out):
    import os

    global LAST_EXEC_NS

    token = np.asarray(token)
    hidden = np.asarray(hidden, dtype=np.float32)
    emb = np.asarray(emb, dtype=np.float32)
    w_ih = np.asarray(w_ih, dtype=np.float32)
    w_hh = np.asarray(w_hh, dtype=np.float32)
    b_ih = np.asarray(b_ih, dtype=np.float32)
    b_hh = np.asarray(b_hh, dtype=np.float32)
    W_out = np.asarray(W_out, dtype=np.float32)
    b_out = np.asarray(b_out, dtype=np.float32)

    tok = int(token.reshape(-1)[0])
    x_row = emb[tok]                       # [1024], pre-relu (relu on device)
    h_row = hidden.reshape(NHID)

    xe = np.empty((P, F), np.float32)
    xe[:, :NHID] = x_row
    xe[:, NHID] = 1.0
    he = np.empty((P, F), np.float32)
    he[:, :NHID] = h_row
    he[:, NHID] = 1.0

    # GRU weight shards: [core][128 units, 3 gates, 1024+1]
    wih4 = w_ih.reshape(3, N_CORES, HP, NHID)
    whh4 = w_hh.reshape(3, N_CORES, HP, NHID)
    bih3 = b_ih.reshape(3, N_CORES, HP)
    bhh3 = b_hh.reshape(3, N_CORES, HP)

    # Output weights, vocab padded to 51200, bias packed in col 1024
    Wp = np.zeros((VPAD, F), np.float32)
    Wp[:NOUT, :NHID] = W_out
    Wp[:NOUT, NHID] = b_out
    Wp[NOUT:, NHID] = NEG_BIG
    wout_all = Wp.reshape(N_CORES, P, SLOTS, F)

    h_slices = h_row.reshape(N_CORES, HP)

    if "nc" not in _CACHE:
        _CACHE["nc"] = _build()
    nc = _CACHE["nc"]

    in_maps = []
    for c in range(N_CORES):
        wih_c = np.concatenate(
            [wih4[:, c].transpose(1, 0, 2), bih3[:, c].T[:, :, None]], axis=2
        )
        whh_c = np.concatenate(
            [whh4[:, c].transpose(1, 0, 2), bhh3[:, c].T[:, :, None]], axis=2
        )
        in_maps.append(
            {
                "xe": xe,
                "he": he,
                "hsl": np.ascontiguousarray(h_slices[c][:, None]),
                "wih": np.ascontiguousarray(wih_c),
                "whh": np.ascontiguousarray(whh_c),
                "wout": np.ascontiguousarray(wout_all[c]),
            }
        )

    trace = os.environ.get("KERNEL_TRACE", "0") == "1"
    res = bass_utils.run_bass_kernel_spmd(
        nc, in_maps, core_ids=list(range(N_CORES)), trace=trace
    )
    LAST_EXEC_NS = res.exec_time_ns

    logp = np.concatenate(
        [res.results[c]["logp"].reshape(-1) for c in range(N_CORES)]
    )[:NOUT].reshape(1, NOUT)
    h_full = np.concatenate(
        [res.results[c]["hout"][:, 0] for c in range(N_CORES)]
    ).reshape(1, 1, NHID)
    return logp, h_full
